# revision 25
# baseline (speedup 1.0000x reference)
"""CapsNet dynamic-routing layer on 8 Trainium2 NeuronCores — v4.

Strategy
--------
Routes sharded 8 ways (64/core). Phase A is W-DMA-bound, so W is read as
3 bytes/elem: fp16 hi + fp8e4m3 lo of the residual pre-scaled by 2^12
(host-side; rel err of the 3-byte W is 4.9e-6, measured). u_hat via
3-pass TensorE matmuls per tile: Wh*xh + Wh*xl + W8*(xh/2^12) — mixed
fp16(stationary) x fp8(moving) matmul verified exact on HW. psum
evacuated to a single fp32 u4 [128,(rj,b),g,co] copy (routing needs
~full fp32: fp16/bf16 anywhere in the loop fails the 2e-2 bar by 3-10x,
measured). s0 = sum_r u/C accumulated on DVE into SBUF, reduced over
rj-strips by tiny delta matmuls; a SINGLE 256KB AllReduce (RDH) fires at
the end of Phase A. A full-size 256KB warmup AllReduce at kernel start
absorbs first-collective + first-RDH setup (s0's RDH measured 28us cold
vs 13-15us warm).

Phase B (per routing iteration): all big elementwise work runs fp32 on
DVE at 1x (fp16/bf16 and GpSimd offload both measured slower or wrong),
2 r-groups per DVE instruction; the r-reduction of s goes to TensorE as
float32r delta matmuls (1 cyc/row at N=512 vs fp32's 4; costs ~4e-3
final rel err vs the 2e-2 bar). s is AllReduced across cores through
DRAM (fp32). Output is written in the packed (c-quarter, batch) layout
and unpacked host-side.
"""
import sys

sys.path.insert(0, "/opt/trn_rl_repo")

import numpy as np

import concourse.bass as bass
import concourse.tile as tile
from concourse import mybir
from concourse.bass_utils import run_bass_kernel_spmd

F16 = mybir.dt.float16
F32 = mybir.dt.float32
F32R = mybir.dt.float32r
F8 = mybir.dt.float8e4
AX = mybir.AxisListType
OP = mybir.AluOpType
AF = mybir.ActivationFunctionType

NCORES = 8
B, R, C, O, I = 32, 512, 32, 64, 128
CO = C * O                # 2048
RL = R // NCORES          # 64 routes per core
J = 4                     # col-strips (rj)
G = RL // J               # 16 r-groups
NQ = 4                    # co chunks
Q = CO // NQ              # 512
EPS = 1e-8

USE_F32R = True           # float32r delta matmuls (4x faster than fp32)
SPLIT_WDMA = True         # alternate W-tile DMAs across SP/ACT HW queues
WBUFS = 6                 # W tile double-buffer depth
LO_SCALE = 4096.0         # W-lo fp8 pre-scale (2^12)

_cache = {}


def _legalize_install(nc):
    """This walrus build accepts at most one sync wait per instruction and
    none on Matmult; hoist extras onto standalone EventSemaphore ops."""
    import json
    from concourse import mybir as _mb

    def legalize(raw: bytes) -> bytes:
        d = json.loads(raw)
        ctr = 0
        for f in d.get("functions", []):
            for blk in f.get("blocks", []):
                out = []
                for ins in blk.get("instructions", []):
                    si = ins.get("sync_info")
                    waits = (si or {}).get("on_wait") or []
                    keep = 0 if ins.get("opcode") in ("Matmult", "Ldweights") else 1
                    if len(waits) > keep:
                        nh = len(waits) - keep
                        for w in waits[:nh]:
                            ctr += 1
                            out.append({
                                "debug": ins.get("debug", 0),
                                "engine": ins["engine"],
                                "ins": [], "outs": [],
                                "name": f"lgl_wait_{ctr}",
                                "opcode": "EventSemaphore",
                                "sync_info": {"on_update": [], "on_wait": [w]},
                            })
                        si["on_wait"] = waits[nh:]
                    out.append(ins)
                blk["instructions"] = out
        return json.dumps(d).encode()

    nc.to_json_bytes = lambda: legalize(_mb.module_to_json_bytes(nc.m))
    return nc


def _build():
    nc = bass.Bass(trn_type="TRN2", target_bir_lowering=False, debug=False,
                   num_devices=NCORES)

    def mmdt(ap):
        return ap.bitcast(F32R) if USE_F32R else ap

    d_xh = nc.dram_tensor("xh", [I, RL, B], F16, kind="ExternalInput").ap()
    d_xl = nc.dram_tensor("xl", [I, RL, B], F16, kind="ExternalInput").ap()
    d_xh12 = nc.dram_tensor("xh12", [I, RL, B], F16,
                            kind="ExternalInput").ap()
    # prepacked W: per (g,q) tile, fp16 hi [I,(j,Q)] + scaled fp8 lo [I,(j,Q)]
    d_Wh = nc.dram_tensor("Wh", [G * NQ, I, J * Q], F16,
                          kind="ExternalInput").ap()
    d_W8 = nc.dram_tensor("W8", [G * NQ, I, J * Q], F8,
                          kind="ExternalInput").ap()
    d_d0 = nc.dram_tensor("delta_s0", [128, B], F32, kind="ExternalInput").ap()
    d_d1 = nc.dram_tensor("delta_1", [128, B], F32, kind="ExternalInput").ap()
    d_vout = nc.dram_tensor("v_out", [128, 512], F32, kind="ExternalOutput").ap()

    # full-size warmup collective: absorbs both first-collective setup AND
    # the RDH algorithm's first-use cost (s0's RDH measured 28us vs 13-15us
    # for subsequent identical ones)
    d_wa = nc.dram_tensor("warm_a", [B, CO], F32).ap()
    d_wb = nc.dram_tensor("warm_b", [B, CO], F32, addr_space="Shared").ap()
    d_sb = [nc.dram_tensor(f"s_bounce{t}", [B, CO], F32).ap()
            for t in range(3)]
    d_sr = [nc.dram_tensor(f"s_red{t}", [B, CO], F32,
                           addr_space="Shared").ap() for t in range(3)]
    d_vdr = [nc.dram_tensor(f"v_dr{t}", [128, Q], F32).ap() for t in range(2)]

    groups = [list(range(NCORES))]

    with tile.TileContext(nc) as tc:
        with tc.tile_pool(name="const", bufs=1) as cpool, \
             tc.tile_pool(name="upool", bufs=1) as upool:

            t_d0 = cpool.tile([128, B], F32, tag="d0")
            t_d1 = cpool.tile([128, B], F32, tag="d1")
            nc.sync.dma_start(t_d0[:], d_d0)
            nc.sync.dma_start(t_d1[:], d_d1)
            # fp32r-rounded copy of d1 (0/1 values — exact); walrus requires
            # fp32r matmult operands to come from an fp32r-rounding producer
            t_d1r = cpool.tile([128, B], F32, tag="d1r")
            nc.vector.tensor_copy(t_d1r[:].bitcast(F32R), t_d1[:])
            t_eps = cpool.tile([128, 1], F32, tag="eps")
            nc.gpsimd.memset(t_eps[:], EPS)
            t_warm = cpool.tile([B, CO], F32, tag="warm")
            nc.gpsimd.memset(t_warm[:], EPS)
            nc.sync.dma_start(d_wa, t_warm[:])
            nc.gpsimd.collective_compute(
                "AllReduce", OP.add, replica_groups=groups,
                ins=[d_wa.opt()], outs=[d_wb.opt()])

            t_u = upool.tile([128, G, CO], F32, tag="u")   # 128 KiB/part
            u4 = t_u[:].rearrange("p g (c o) -> p g c o", c=C)
            t_sbounce = upool.tile([B, CO], F32, tag="sbounce")

            # ---- Phase A ----
            with tc.tile_pool(name="xpool", bufs=1) as xpool, \
                 tc.tile_pool(name="wpool", bufs=WBUFS) as wpool, \
                 tc.tile_pool(name="prodps", bufs=3, space="PSUM") as prodps:
                t_acc = xpool.tile([128, NQ, Q], F32, tag="s0acc")
                t_xh = xpool.tile([I, RL * B], F16, tag="xh")
                t_xl = xpool.tile([I, RL * B], F16, tag="xl")
                t_xh12 = xpool.tile([I, RL * B], F16, tag="xh12")
                nc.sync.dma_start(
                    t_xh[:].rearrange("i (r b) -> i r b", r=RL), d_xh)
                nc.sync.dma_start(
                    t_xl[:].rearrange("i (r b) -> i r b", r=RL), d_xl)
                nc.scalar.dma_start(
                    t_xh12[:].rearrange("i (r b) -> i r b", r=RL), d_xh12)

                for q in range(NQ):
                    for g in range(G):
                        ti = q * G + g
                        wh = wpool.tile([I, J, Q], F16, tag="wh")
                        w8 = wpool.tile([I, J, Q], F8, tag="w8")
                        e0, e1 = ((nc.sync, nc.scalar) if
                                  (not SPLIT_WDMA or ti % 2 == 0) else
                                  (nc.scalar, nc.sync))
                        e0.dma_start(wh[:].rearrange("i j q -> i (j q)"),
                                     d_Wh[ti])
                        e1.dma_start(w8[:].rearrange("i j q -> i (j q)"),
                                     d_W8[ti])
                        pp = prodps.tile([128, Q], F32, tag="prod")
                        for j in range(J):
                            r = J * g + j
                            sxh = t_xh[:, r * B:(r + 1) * B]
                            sxl = t_xl[:, r * B:(r + 1) * B]
                            sxh12 = t_xh12[:, r * B:(r + 1) * B]
                            tp = (0, 32 * j)
                            ppj = pp[32 * j:32 * (j + 1), :]
                            nc.tensor.matmul(ppj, sxh, wh[:, j, :],
                                             start=True, stop=False,
                                             tile_position=tp)
                            nc.tensor.matmul(ppj, sxl, wh[:, j, :],
                                             start=False, stop=False,
                                             tile_position=tp)
                            nc.tensor.matmul(ppj, sxh12, w8[:, j, :],
                                             start=False, stop=True,
                                             tile_position=tp)
                        useg = t_u[:, g, Q * q:Q * q + Q]
                        nc.scalar.copy(useg, pp[:])
                        accq = t_acc[:, q, :]
                        if g == 0:
                            nc.vector.tensor_copy(accq, pp[:])
                        else:
                            nc.vector.tensor_add(accq, accq, pp[:])
                    ps0 = prodps.tile([B, Q], F32, tag="s0q", bufs=1,
                                      name=f"s0q{q}")
                    nc.tensor.matmul(ps0[:], t_d0[:], t_acc[:, q, :],
                                     start=True, stop=True)
                    nc.scalar.copy(t_sbounce[:, Q * q:Q * q + Q], ps0[:])
                # single full-size s0 AllReduce (RDH) at end of Phase A
                nc.sync.dma_start(d_sb[0], t_sbounce[:])
                nc.gpsimd.collective_compute(
                    "AllReduce", OP.add, replica_groups=groups,
                    ins=[d_sb[0].opt()], outs=[d_sr[0].opt()])

            # ---- Phase B ----
            with tc.tile_pool(name="iter", bufs=1) as ip, \
                 tc.tile_pool(name="tmp", bufs=2) as tp_pool, \
                 tc.tile_pool(name="sps", bufs=1, space="PSUM") as sps:

                t_vrep = ip.tile([128, CO], F32, tag="vrep")
                t_b = ip.tile([128, G, C], F32, tag="bij")
                t_a = ip.tile([128, G, C], F32, tag="aij")
                t_e = ip.tile([128, G, C], F32, tag="eij")
                t_c = ip.tile([128, G, C], F32, tag="cij")
                t_mx = ip.tile([128, G], F32, tag="mx")
                t_rs = ip.tile([128, G], F32, tag="rs")
                t_spk = ip.tile([128, NQ * C // 4 * O // NQ], F32, tag="spk")
                t_sq = ip.tile([128, C // 4 * O], F32, tag="sqt")
                t_rt = ip.tile([128, C // 4 * O], F32, tag="rt")
                t_p1 = ip.tile([128, C // 4 * O], F32, tag="p1t")
                t_vpk = ip.tile([128, C // 4 * O], F32, tag="vpk")

                def allreduce(t):
                    nc.sync.dma_start(d_sb[t], t_sbounce[:])
                    nc.gpsimd.collective_compute(
                        "AllReduce", OP.add,
                        replica_groups=groups,
                        ins=[d_sb[t].opt()], outs=[d_sr[t].opt()])

                def squash(t):
                    """d_sr[t] -> packed v in t_vpk; partitions (cq, b),
                    free (c', o) with c = cq*8 + c'."""
                    srv = d_sr[t].rearrange("b (cq f) -> cq b f", cq=4)
                    for cq in range(4):
                        nc.sync.dma_start(
                            t_spk[32 * cq:32 * (cq + 1), :], srv[cq])
                    nc.scalar.square(t_sq[:], t_spk[:])
                    nc.scalar.activation(t_rt[:], t_sq[:], AF.Sqrt,
                                         bias=t_eps[:])
                    nc.vector.tensor_scalar_add(t_p1[:], t_sq[:], 1.0)
                    nc.vector.tensor_mul(t_rt[:], t_rt[:], t_p1[:])
                    nc.vector.reciprocal(t_rt[:], t_rt[:])
                    nc.vector.tensor_mul(t_sq[:], t_sq[:], t_spk[:])
                    nc.vector.tensor_mul(t_vpk[:], t_sq[:], t_rt[:])

                def vdist(dst):
                    """vpk -> DRAM (packed, 1 DMA) -> 4 strip reads."""
                    nc.sync.dma_start(dst, t_vpk[:])
                    dv = dst.rearrange("(cq b) f -> b cq f", cq=4)
                    for j in range(J):
                        nc.sync.dma_start(
                            t_vrep[32 * j:32 * (j + 1), :]
                            .rearrange("b (cq f) -> b cq f", cq=4), dv)

                def a_pass(first):
                    """a[p,g,c] = sum_o u4[p,g,c,o] * vrep[p,c,o]; 2 r-groups
                    per DVE instruction (fewer per-instruction overheads)."""
                    dst = t_b if first else t_a
                    vb = t_vrep[:].unsqueeze(1).broadcast_to([128, 2, CO])
                    for g2 in range(G // 2):
                        g = 2 * g2
                        tmp = tp_pool.tile([128, 2, CO], F32, tag="tmp",
                                           bufs=2)
                        nc.vector.tensor_mul(tmp[:], t_u[:, g:g + 2, :], vb)
                        nc.vector.tensor_reduce(
                            dst[:, g:g + 2, :],
                            tmp[:].rearrange("p g (c o) -> p g c o", c=C),
                            axis=AX.X, op=OP.add)
                    if not first:
                        nc.vector.tensor_add(t_b[:], t_b[:], t_a[:])

                def softmax():
                    nc.vector.tensor_reduce(t_mx[:], t_b[:], axis=AX.X,
                                            op=OP.max)
                    mxb = t_mx[:].unsqueeze(2).broadcast_to([128, G, C])
                    nc.vector.tensor_sub(t_e[:], t_b[:], mxb)
                    nc.scalar.activation(t_e[:], t_e[:], AF.Exp)
                    nc.vector.tensor_reduce(t_rs[:], t_e[:], axis=AX.X,
                                            op=OP.add)
                    nc.vector.reciprocal(t_rs[:], t_rs[:])
                    rsb = t_rs[:].unsqueeze(2).broadcast_to([128, G, C])
                    nc.vector.tensor_tensor(t_c[:], t_e[:], rsb, OP.mult)

                def s_pass(t):
                    """s[b,co] = sum_{rj,g} c[p,g,c] * u4[p,g,c,o]; all 16
                    g-groups reduced by TensorE float32r delta matmuls; 2
                    r-groups per DVE mult."""
                    sq = [sps.tile([B, Q], F32, tag=f"sq{q}",
                                   name=f"sq{q}_{t}") for q in range(NQ)]
                    dmm = t_d1r[:] if USE_F32R else t_d1[:]
                    for g2 in range(G // 2):
                        g = 2 * g2
                        tt = tp_pool.tile([128, 2, CO], F32, tag="tmp",
                                          bufs=2)
                        cb = t_c[:, g:g + 2, :].unsqueeze(3) \
                            .broadcast_to([128, 2, C, O])
                        nc.vector.tensor_tensor(
                            mmdt(tt[:]).rearrange("p g (c o) -> p g c o",
                                                  c=C),
                            u4[:, g:g + 2, :, :], cb, OP.mult)
                        for gg in range(2):
                            for q in range(NQ):
                                nc.tensor.matmul(
                                    sq[q][:], mmdt(dmm),
                                    mmdt(tt[:, gg, Q * q:Q * q + Q]),
                                    start=(g + gg == 0),
                                    stop=(g + gg == G - 1))
                    for q in range(NQ):
                        nc.scalar.copy(t_sbounce[:, Q * q:Q * q + Q], sq[q][:])
                    allreduce(t)

                # ---- iteration 0 (s0 AllReduced at end of Phase A) ----
                squash(0)
                vdist(d_vdr[0])
                a_pass(first=True)

                softmax()
                s_pass(1)
                squash(1)
                vdist(d_vdr[1])
                a_pass(first=False)

                softmax()
                s_pass(2)
                squash(2)
                nc.sync.dma_start(d_vout, t_vpk[:])

    _legalize_install(nc)
    return nc


def _prep_inputs(x, W):
    import ml_dtypes
    x_t = np.ascontiguousarray(x.transpose(2, 1, 0))          # [I, R, B]
    xh = x_t.astype(np.float16)
    xl = (x_t - xh.astype(np.float32)).astype(np.float16)
    xh12 = (xh.astype(np.float32) / LO_SCALE).astype(np.float16)
    W_t = np.ascontiguousarray(W.transpose(0, 3, 1, 2)).reshape(R, I, CO)
    d0 = np.tile(np.eye(B, dtype=np.float32) / C, (J, 1))
    d1 = np.tile(np.eye(B, dtype=np.float32), (J, 1))
    in_maps = []
    for k in range(NCORES):
        rk = slice(RL * k, RL * (k + 1))
        Wc = W_t[rk]                                          # [64, I, CO]
        # [g, j, i, q, Q] -> [q, g, i, j, Q]  (q-major tile order)
        Wc = Wc.reshape(G, J, I, NQ, Q).transpose(3, 0, 2, 1, 4)
        Wh = Wc.astype(np.float16)
        W8 = ((Wc - Wh.astype(np.float32)) * LO_SCALE) \
            .astype(ml_dtypes.float8_e4m3)
        in_maps.append({
            "xh": np.ascontiguousarray(xh[:, rk, :]),
            "xl": np.ascontiguousarray(xl[:, rk, :]),
            "xh12": np.ascontiguousarray(xh12[:, rk, :]),
            "Wh": np.ascontiguousarray(Wh.reshape(G * NQ, I, J * Q)),
            "W8": np.ascontiguousarray(W8.reshape(G * NQ, I, J * Q)),
            "delta_s0": d0, "delta_1": d1,
        })
    return in_maps


def kernel(x: np.ndarray, W: np.ndarray, **run_kwargs) -> np.ndarray:
    if "nc" not in _cache:
        _cache["nc"] = _build()
    nc = _cache["nc"]
    in_maps = _prep_inputs(np.asarray(x), np.asarray(W))
    res = run_bass_kernel_spmd(nc, in_maps, core_ids=list(range(NCORES)),
                               **run_kwargs)
    vp = res.results[0]["v_out"].reshape(4, B, C // 4, O)
    v = np.ascontiguousarray(vp.transpose(1, 0, 2, 3)).reshape(B, C, O, 1) \
        .astype(np.float32)
    if run_kwargs:
        _cache["last_results"] = res
    return v
